# revision 26
# baseline (speedup 1.0000x reference)
"""BitNet ternary layer on 8 trn2 NeuronCores.

y[b,s,o] = sum_i x[b,s,i] * tq(w)[o,i],  tq(w) = sign(w) * (|w| > 0.7*mean|w|)

Distribution: data-parallel over the batch dim — core c computes the full
matmul for batch c. Host-side prep is layout only: w is transposed once to
wT [in, out] and each core's x slice to xT [in, tok], so the contraction
dim lands on SBUF partitions with no on-device transposes.

Per core:
  A) partial absmean over 1/8 of w (its wpart slice) via Act Abs+accum_out,
     then a cross-core AllReduce of the pre-scaled [1,1] partial -> exact
     global threshold t without a serial 67MB read.
  B) xT is cast-loaded fp32->bf16 straight into SBUF (SWDGE cast DMA) and
     stays resident (128KB/partition).
  C) wT is streamed in 512-wide out-column stripes and quantized directly
     into the matmul rhs layout: q2 = Sign(w-t) + Sign(w+t) in {-2,0,2}
     (two Act passes + one DVE add; exact in bf16), double buffered ahead
     of the TensorEngine. 4096 bf16 matmuls (K=128, M=128, N=512)
     accumulate over 32 k-tiles into PSUM; Act drains PSUM->SBUF with
     scale=0.5 (exact) and y tiles DMA out in the natural [tok, out]
     layout.
Pass order: stripe 0 runs tokens 0..1023 first (so matmul starts before all
of x has landed), stripes 1..7 run all tokens, then stripe 0's second half.
"""
import copy
import sys

sys.path.insert(0, '/opt/trn_rl_repo')

import numpy as np

import concourse.bass as bass
from concourse import mybir
from concourse.tile import TileContext
from concourse.vector_clock import ScopedClock
from concourse.bass_utils import run_bass_kernel_spmd

# ---------------------------------------------------------------------------
# Workarounds for this container's walrus build rejecting sem-waits attached
# to several instruction structs (CTRL/NoOp/Drain/DMA-transpose): emit the
# TileContext exit waits as standalone wait_ge instructions, and post-process
# the module to hoist every immediate sem-wait onto its own single-wait
# InstEventSemaphore (same engine, same program position -> same semantics).
# ---------------------------------------------------------------------------


def _patched_drain_and_barrier(self, tick_clock, wait_clock):
    probe = self.nc.sync.nop()
    wait_clock.add_sem_waits(probe.ins, ScopedClock({None: tick_clock.global_clock}))
    si = probe.ins.sync_info
    waits = list(si.on_wait) if si is not None else []
    if waits:
        probe.ins.sync_info = mybir.SyncInfo(on_wait=[], on_update=list(si.on_update))
        for w in waits:
            self.nc.sync.wait_ge(bass.SemaphoreHandle(w.ant_name, w.id), w.wait_value)
    self.nc.sync.drain()
    self.nc.all_engine_barrier()
    assert self.sems is not None
    popped = self.nc._tile_sem_poison_stack.pop()
    assert popped is self._sem_poison
    self.nc.clear_and_free_semaphores(list(self.sems.allocated().values()))
    self.nc.all_engine_barrier()


TileContext._drain_and_barrier = _patched_drain_and_barrier

_ctr = [0]


def _hoist_waits(nc):
    new_module = copy.replace(nc.m, functions=[])
    for function in nc.m.functions:
        new_function = copy.replace(function, blocks=[])
        new_function.set_allocations_from_list(function.allocations)
        for block in function.blocks:
            new_insts = []
            for inst in block.instructions:
                si = inst.sync_info
                if si is not None and not isinstance(inst, mybir.InstEventSemaphore):
                    imm = [w for w in si.on_wait if w.wait_reg is None]
                    if imm:
                        reg = [w for w in si.on_wait if w.wait_reg is not None]
                        for w in imm:
                            _ctr[0] += 1
                            ev = mybir.InstEventSemaphore(
                                name=f"HW-{_ctr[0]}", ins=[], outs=[])
                            ev.engine = inst.engine
                            ev.sync_info = mybir.SyncInfo(on_wait=[w], on_update=[])
                            new_insts.append(ev)
                        inst.sync_info = mybir.SyncInfo(
                            on_wait=reg, on_update=list(si.on_update))
                new_insts.append(inst)
            new_block = copy.replace(block, instructions=new_insts)
            new_function.blocks.append(new_block)
        new_module.functions.append(new_function)
    nc.m = new_module
    return nc


# ---------------------------------------------------------------------------
# Problem shapes (hardcoded per spec)
# ---------------------------------------------------------------------------
B = 8            # batch -> one per core
S = 2048         # tokens per core
I = 4096         # in features (contraction)
O = 4096         # out features
P = 128
NK = I // P      # 32 k-tiles
OC = 512         # out-column stripe width (one PSUM bank at fp32)
NOC = O // OC    # 8 stripes
KG = 2           # k-tiles per quantized rhs tile
NG = NK // KG    # 8 rhs tiles per stripe
NS = S // P      # 16 token tiles
WPR = I // B     # 512 wT rows per core for the partial absmean

def build_program():
    fp32 = mybir.dt.float32
    bf16 = mybir.dt.bfloat16

    nc = bass.Bass(num_devices=B)
    xT_in = nc.declare_dram_parameter("xT", [I, S], fp32, isOutput=False)
    wT_in = nc.declare_dram_parameter("wT", [I, O], fp32, isOutput=False)
    wp_in = nc.declare_dram_parameter("wp", [WPR, O], fp32, isOutput=False)
    y_out = nc.declare_dram_parameter("y", [S, O], fp32, isOutput=True)

    with TileContext(nc) as tc:
        with (
            tc.tile_pool(name="dram", bufs=1, space="DRAM") as dram,
            tc.tile_pool(name="singles", bufs=1) as singles,
            tc.tile_pool(name="xT", bufs=1) as xT_pool,
            tc.tile_pool(name="wf", bufs=2) as wf_pool,
            tc.tile_pool(name="qtmp", bufs=4) as qtmp_pool,
            tc.tile_pool(name="ysb", bufs=3) as ysb_pool,
            tc.tile_pool(name="psA", bufs=1, space="PSUM") as psum1,
            tc.tile_pool(name="ps", bufs=6, space="PSUM") as psum_pool,
        ):
            cc_in = dram.tile([1, 1], fp32, name="ccin")
            cc1 = dram.tile([1, 1], fp32, name="cc1")
            cc2 = dram.tile([1, 1], fp32, name="cc2")
            cc_out = dram.tile([1, 1], fp32, name="ccout")

            xTt = xT_pool.tile([P, NK, S], bf16)
            partials = singles.tile([P, 8], fp32, tag="partials")
            part1 = singles.tile([P, 1], fp32, tag="part1")
            ones = singles.tile([P, 1], fp32, tag="ones")
            ssum = singles.tile([1, 1], fp32, tag="ssum")
            t_b = singles.tile([P, 1], fp32, tag="tb")
            nt_b = singles.tile([P, 1], fp32, tag="ntb")

            # ---- phase A: threshold t = 0.7 * mean|w| via partial absmean.
            # Abs+sum runs on Act (activation accum_out sums the free dim in
            # one pass; DVE tensor_reduce measures 5.1us/chunk with the SBUF
            # errata). wpart DMAs alternate sync/scalar HWDGE rings. The
            # 0.7/(O*I) scale is applied BEFORE the exchange (sum is linear),
            # and the exchange is an AllGather (N-1 ring hops) + local sum
            # rather than an AllReduce (2(N-1) hops) -- latency, not BW.
            with (
                tc.tile_pool(name="phain", bufs=3) as phain,
                tc.tile_pool(name="phascr", bufs=2) as phascr,
            ):
                for jj in range(2 * (WPR // P)):
                    j, hh = jj // 2, jj % 2
                    wa = phain.tile([P, O // 2], fp32)
                    eng = nc.sync if jj % 2 == 0 else nc.scalar
                    eng.dma_start(
                        out=wa[:],
                        in_=wp_in[j * P:(j + 1) * P,
                                  hh * (O // 2):(hh + 1) * (O // 2)])
                    ascr = phascr.tile([P, O // 2], fp32)
                    nc.scalar.activation(
                        ascr[:], wa[:], mybir.ActivationFunctionType.Abs,
                        accum_out=partials[:, jj:jj + 1])
                nc.vector.tensor_reduce(
                    part1[:], partials[:], axis=mybir.AxisListType.X,
                    op=mybir.AluOpType.add)
                nc.vector.memset(ones[:], 1.0)
                tsum = psum1.tile([1, 1], fp32)
                nc.tensor.matmul(tsum[:], lhsT=part1[:], rhs=ones[:],
                                 start=True, stop=True)
                nc.scalar.activation(ssum[:], tsum[:],
                                     mybir.ActivationFunctionType.Copy,
                                     scale=0.7 / float(O * I))
                nc.sync.dma_start(out=cc_in[:], in_=ssum[:])
                # Hypercube AllReduce: three 2-core stages (each a 2-hop
                # ring) instead of one 8-core ring (14 hops). Every core
                # ends with the global sum.
                nc.gpsimd.collective_compute(
                    "AllReduce", mybir.AluOpType.add,
                    replica_groups=[[0, 1], [2, 3], [4, 5], [6, 7]],
                    ins=[cc_in[:].opt()], outs=[cc1[:].opt()])
                nc.gpsimd.collective_compute(
                    "AllReduce", mybir.AluOpType.add,
                    replica_groups=[[0, 2], [1, 3], [4, 6], [5, 7]],
                    ins=[cc1[:].opt()], outs=[cc2[:].opt()])
                nc.gpsimd.collective_compute(
                    "AllReduce", mybir.AluOpType.add,
                    replica_groups=[[0, 4], [1, 5], [2, 6], [3, 7]],
                    ins=[cc2[:].opt()], outs=[cc_out[:].opt()])

            # ---- x: cast-load fp32->bf16, deferred behind the collective
            # issue so phase A's wpart read gets full HBM bandwidth.
            for h in range(2):
                for k in range(NK):
                    nc.gpsimd.dma_start(
                        out=xTt[:, k, h * 1024:(h + 1) * 1024],
                        in_=xT_in[k * P:(k + 1) * P, h * 1024:(h + 1) * 1024])

            # cc_out holds t; broadcast to [P,1] (stride-0 partition read).
            # On the scalar ring: Act is idle until t exists anyway.
            t_bcast_ap = bass.AP(
                tensor=cc_out.tensor, offset=cc_out.offset,
                ap=[[0, P], [1, 1]])
            nc.scalar.dma_start(out=t_b[:], in_=t_bcast_ap)
            nc.vector.tensor_scalar_mul(nt_b[:], t_b[:], -1.0)

            # ---- main: 9 passes (stripe 0 split around the x half-load)
            with tc.tile_pool(name="wq", bufs=20) as wq_pool:
                for p_i in range(NOC + 1):
                    oc = p_i if p_i < NOC else 0
                    if p_i == 0:
                        s_list = range(0, NS // 2)
                    elif p_i == NOC:
                        s_list = range(NS // 2, NS)
                    else:
                        s_list = range(NS)

                    wq_tiles = []
                    for g in range(NG):
                        wf = wf_pool.tile([P, KG * OC], fp32, tag="wf")
                        src = bass.AP(
                            tensor=wT_in,
                            offset=(g * KG * P) * O + oc * OC,
                            ap=[[O, P], [P * O, KG], [1, OC]])
                        nc.sync.dma_start(out=wf[:], in_=src)
                        # q2 = Sign(w-t) + Sign(w+t) in {-2,0,2}; the matmul
                        # result is scaled by 0.5 in the PSUM drain (exact).
                        s1 = qtmp_pool.tile([P, KG * OC], bf16, tag="s1")
                        s2 = qtmp_pool.tile([P, KG * OC], bf16, tag="s2")
                        wq = wq_pool.tile([P, KG * OC], bf16, tag="wq")
                        nc.scalar.sign(s1[:], wf[:], bias=nt_b[:])
                        nc.scalar.sign(s2[:], wf[:], bias=t_b[:])
                        nc.vector.tensor_add(wq[:], s1[:], s2[:])
                        wq_tiles.append(wq)

                    for s in s_list:
                        ps = psum_pool.tile([P, OC], fp32)
                        for k in range(NK):
                            nc.tensor.matmul(
                                ps[:],
                                lhsT=xTt[:, k, s * P:(s + 1) * P],
                                rhs=wq_tiles[k // KG][
                                    :, (k % KG) * OC:(k % KG + 1) * OC],
                                start=(k == 0), stop=(k == NK - 1))
                        ob = ysb_pool.tile([P, OC], fp32)
                        nc.scalar.activation(
                            ob[:], ps[:], mybir.ActivationFunctionType.Copy,
                            scale=0.5)
                        nc.sync.dma_start(
                            out=y_out[s * P:(s + 1) * P,
                                      oc * OC:(oc + 1) * OC],
                            in_=ob[:])

    _hoist_waits(nc)
    return nc


_program_cache = None


def _get_program():
    global _program_cache
    if _program_cache is None:
        _program_cache = build_program()
    return _program_cache


def run(x, weight, trace=False, trace_cores=None):
    x = np.asarray(x, dtype=np.float32)
    weight = np.asarray(weight, dtype=np.float32)
    assert x.shape == (B, S, I), x.shape
    assert weight.shape == (O, I), weight.shape
    nc = _get_program()
    wT = np.ascontiguousarray(weight.T)           # [I, O]
    in_maps = []
    for c in range(B):
        in_maps.append({
            "xT": np.ascontiguousarray(x[c].T),   # [I, S]
            "wT": wT,
            "wp": wT[c * WPR:(c + 1) * WPR],      # contiguous row slice
        })
    kwargs = {}
    if trace_cores:
        kwargs["trace_cores"] = trace_cores
    res = run_bass_kernel_spmd(nc, in_maps, list(range(B)), trace=trace,
                               **kwargs)
    y = np.stack([res.results[c]["y"] for c in range(B)], axis=0)
    return y, res


def kernel(x, weight):
    y, _ = run(x, weight)
    return y


# revision 28
# speedup vs baseline: 1.0095x; 1.0095x over previous
"""BitNet ternary layer on 8 trn2 NeuronCores.

y[b,s,o] = sum_i x[b,s,i] * tq(w)[o,i],  tq(w) = sign(w) * (|w| > 0.7*mean|w|)

Distribution: data-parallel over the batch dim — core c computes the full
matmul for batch c. Host-side prep is layout only: w is transposed once to
wT [in, out] and each core's x slice to xT [in, tok], so the contraction
dim lands on SBUF partitions with no on-device transposes.

Per core:
  A) partial absmean over 1/8 of w (its wpart slice) via Act Abs+accum_out,
     then a cross-core AllReduce of the pre-scaled [1,1] partial -> exact
     global threshold t without a serial 67MB read.
  B) xT is cast-loaded fp32->bf16 straight into SBUF (SWDGE cast DMA) and
     stays resident (128KB/partition).
  C) wT is streamed in 512-wide out-column stripes and quantized directly
     into the matmul rhs layout: q2 = Sign(w-t) + Sign(w+t) in {-2,0,2}
     (two Act passes + one DVE add; exact in bf16), double buffered ahead
     of the TensorEngine. 4096 bf16 matmuls (K=128, M=128, N=512)
     accumulate over 32 k-tiles into PSUM; Act drains PSUM->SBUF with
     scale=0.5 (exact) and y tiles DMA out in the natural [tok, out]
     layout.
Pass order: stripe 0 runs tokens 0..1023 first (so matmul starts before all
of x has landed), stripes 1..7 run all tokens, then stripe 0's second half.
"""
import copy
import sys

sys.path.insert(0, '/opt/trn_rl_repo')

import numpy as np

import concourse.bass as bass
from concourse import mybir
from concourse.tile import TileContext
from concourse.vector_clock import ScopedClock
from concourse.bass_utils import run_bass_kernel_spmd

# ---------------------------------------------------------------------------
# Workarounds for this container's walrus build rejecting sem-waits attached
# to several instruction structs (CTRL/NoOp/Drain/DMA-transpose): emit the
# TileContext exit waits as standalone wait_ge instructions, and post-process
# the module to hoist every immediate sem-wait onto its own single-wait
# InstEventSemaphore (same engine, same program position -> same semantics).
# ---------------------------------------------------------------------------


def _patched_drain_and_barrier(self, tick_clock, wait_clock):
    probe = self.nc.sync.nop()
    wait_clock.add_sem_waits(probe.ins, ScopedClock({None: tick_clock.global_clock}))
    si = probe.ins.sync_info
    waits = list(si.on_wait) if si is not None else []
    if waits:
        probe.ins.sync_info = mybir.SyncInfo(on_wait=[], on_update=list(si.on_update))
        for w in waits:
            self.nc.sync.wait_ge(bass.SemaphoreHandle(w.ant_name, w.id), w.wait_value)
    self.nc.sync.drain()
    self.nc.all_engine_barrier()
    assert self.sems is not None
    popped = self.nc._tile_sem_poison_stack.pop()
    assert popped is self._sem_poison
    self.nc.clear_and_free_semaphores(list(self.sems.allocated().values()))
    self.nc.all_engine_barrier()


TileContext._drain_and_barrier = _patched_drain_and_barrier

_ctr = [0]


def _hoist_waits(nc):
    new_module = copy.replace(nc.m, functions=[])
    for function in nc.m.functions:
        new_function = copy.replace(function, blocks=[])
        new_function.set_allocations_from_list(function.allocations)
        for block in function.blocks:
            new_insts = []
            for inst in block.instructions:
                si = inst.sync_info
                if si is not None and not isinstance(inst, mybir.InstEventSemaphore):
                    imm = [w for w in si.on_wait if w.wait_reg is None]
                    if imm:
                        reg = [w for w in si.on_wait if w.wait_reg is not None]
                        for w in imm:
                            _ctr[0] += 1
                            ev = mybir.InstEventSemaphore(
                                name=f"HW-{_ctr[0]}", ins=[], outs=[])
                            ev.engine = inst.engine
                            ev.sync_info = mybir.SyncInfo(on_wait=[w], on_update=[])
                            new_insts.append(ev)
                        inst.sync_info = mybir.SyncInfo(
                            on_wait=reg, on_update=list(si.on_update))
                new_insts.append(inst)
            new_block = copy.replace(block, instructions=new_insts)
            new_function.blocks.append(new_block)
        new_module.functions.append(new_function)
    nc.m = new_module
    return nc


# ---------------------------------------------------------------------------
# Problem shapes (hardcoded per spec)
# ---------------------------------------------------------------------------
B = 8            # batch -> one per core
S = 2048         # tokens per core
I = 4096         # in features (contraction)
O = 4096         # out features
P = 128
NK = I // P      # 32 k-tiles
OC = 512         # out-column stripe width (one PSUM bank at fp32)
NOC = O // OC    # 8 stripes
KG = 2           # k-tiles per quantized rhs tile
NG = NK // KG    # 8 rhs tiles per stripe
NS = S // P      # 16 token tiles
WPR = I // B     # 512 wT rows per core for the partial absmean

def build_program():
    fp32 = mybir.dt.float32
    bf16 = mybir.dt.bfloat16

    nc = bass.Bass(num_devices=B)
    xT_in = nc.declare_dram_parameter("xT", [I, S], fp32, isOutput=False)
    wT_in = nc.declare_dram_parameter("wT", [I, O], fp32, isOutput=False)
    wp_in = nc.declare_dram_parameter("wp", [WPR, O], fp32, isOutput=False)
    y_out = nc.declare_dram_parameter("y", [S, O], fp32, isOutput=True)

    with TileContext(nc) as tc:
        with (
            tc.tile_pool(name="dram", bufs=1, space="DRAM") as dram,
            tc.tile_pool(name="singles", bufs=1) as singles,
            tc.tile_pool(name="xT", bufs=1) as xT_pool,
            tc.tile_pool(name="wf", bufs=2) as wf_pool,
            tc.tile_pool(name="qtmp", bufs=4) as qtmp_pool,
            tc.tile_pool(name="ysb", bufs=3) as ysb_pool,
            tc.tile_pool(name="psA", bufs=1, space="PSUM") as psum1,
            tc.tile_pool(name="ps", bufs=6, space="PSUM") as psum_pool,
        ):
            cc_in = dram.tile([1, 1], fp32, name="ccin")
            cc_out = dram.tile([1, 1], fp32, name="ccout")

            xTt = xT_pool.tile([P, NK, S], bf16)
            partials = singles.tile([P, 8], fp32, tag="partials")
            part1 = singles.tile([P, 1], fp32, tag="part1")
            ones = singles.tile([P, 1], fp32, tag="ones")
            ssum = singles.tile([1, 1], fp32, tag="ssum")
            t_b = singles.tile([P, 1], fp32, tag="tb")
            nt_b = singles.tile([P, 1], fp32, tag="ntb")

            # ---- phase A: threshold t = 0.7 * mean|w| via partial absmean.
            # Abs+sum runs on Act (activation accum_out sums the free dim in
            # one pass; DVE tensor_reduce measures 5.1us/chunk with the SBUF
            # errata). wpart DMAs alternate sync/scalar HWDGE rings. The
            # 0.7/(O*I) scale is applied BEFORE the exchange (sum is linear),
            # and the exchange is an AllGather (N-1 ring hops) + local sum
            # rather than an AllReduce (2(N-1) hops) -- latency, not BW.
            with (
                tc.tile_pool(name="phain", bufs=3) as phain,
                tc.tile_pool(name="phascr", bufs=2) as phascr,
            ):
                for jj in range(2 * (WPR // P)):
                    j, hh = jj // 2, jj % 2
                    wa = phain.tile([P, O // 2], fp32)
                    eng = nc.sync if jj % 2 == 0 else nc.scalar
                    eng.dma_start(
                        out=wa[:],
                        in_=wp_in[j * P:(j + 1) * P,
                                  hh * (O // 2):(hh + 1) * (O // 2)])
                    ascr = phascr.tile([P, O // 2], fp32)
                    nc.scalar.activation(
                        ascr[:], wa[:], mybir.ActivationFunctionType.Abs,
                        accum_out=partials[:, jj:jj + 1])
                nc.vector.tensor_reduce(
                    part1[:], partials[:], axis=mybir.AxisListType.X,
                    op=mybir.AluOpType.add)
                nc.vector.memset(ones[:], 1.0)
                tsum = psum1.tile([1, 1], fp32)
                nc.tensor.matmul(tsum[:], lhsT=part1[:], rhs=ones[:],
                                 start=True, stop=True)
                nc.scalar.activation(ssum[:], tsum[:],
                                     mybir.ActivationFunctionType.Copy,
                                     scale=0.7 / float(O * I))
                nc.sync.dma_start(out=cc_in[:], in_=ssum[:])
                # Single 8-core AllReduce. Measured ~95us for 4 bytes, but
                # beats the alternatives: AllGather+local sum ~170us, a
                # 3-stage hypercube of pair-AllReduces ~+55us (per-op fixed
                # cost dominates), and a local full-w absmean ~187us.
                nc.gpsimd.collective_compute(
                    "AllReduce", mybir.AluOpType.add,
                    replica_groups=[list(range(B))],
                    ins=[cc_in[:].opt()], outs=[cc_out[:].opt()])

            # ---- x: cast-load fp32->bf16, deferred behind the collective
            # issue so phase A's wpart read gets full HBM bandwidth.
            for h in range(2):
                for k in range(NK):
                    nc.gpsimd.dma_start(
                        out=xTt[:, k, h * 1024:(h + 1) * 1024],
                        in_=xT_in[k * P:(k + 1) * P, h * 1024:(h + 1) * 1024])

            # cc_out holds t; broadcast to [P,1] (stride-0 partition read).
            # On the scalar ring: Act is idle until t exists anyway.
            t_bcast_ap = bass.AP(
                tensor=cc_out.tensor, offset=cc_out.offset,
                ap=[[0, P], [1, 1]])
            nc.scalar.dma_start(out=t_b[:], in_=t_bcast_ap)
            nc.vector.tensor_scalar_mul(nt_b[:], t_b[:], -1.0)

            # ---- main: 9 passes (stripe 0 split around the x half-load)
            with tc.tile_pool(name="wq", bufs=20) as wq_pool:
                for p_i in range(NOC + 1):
                    oc = p_i if p_i < NOC else 0
                    if p_i == 0:
                        s_list = range(0, NS // 2)
                    elif p_i == NOC:
                        s_list = range(NS // 2, NS)
                    else:
                        s_list = range(NS)

                    wq_tiles = []
                    for g in range(NG):
                        wf = wf_pool.tile([P, KG * OC], fp32, tag="wf")
                        src = bass.AP(
                            tensor=wT_in,
                            offset=(g * KG * P) * O + oc * OC,
                            ap=[[O, P], [P * O, KG], [1, OC]])
                        nc.sync.dma_start(out=wf[:], in_=src)
                        # q2 = Sign(w-t) + Sign(w+t) in {-2,0,2}; the matmul
                        # result is scaled by 0.5 in the PSUM drain (exact).
                        s1 = qtmp_pool.tile([P, KG * OC], bf16, tag="s1")
                        s2 = qtmp_pool.tile([P, KG * OC], bf16, tag="s2")
                        wq = wq_pool.tile([P, KG * OC], bf16, tag="wq")
                        nc.scalar.sign(s1[:], wf[:], bias=nt_b[:])
                        nc.scalar.sign(s2[:], wf[:], bias=t_b[:])
                        nc.vector.tensor_add(wq[:], s1[:], s2[:])
                        wq_tiles.append(wq)

                    for s in s_list:
                        ps = psum_pool.tile([P, OC], fp32)
                        for k in range(NK):
                            nc.tensor.matmul(
                                ps[:],
                                lhsT=xTt[:, k, s * P:(s + 1) * P],
                                rhs=wq_tiles[k // KG][
                                    :, (k % KG) * OC:(k % KG + 1) * OC],
                                start=(k == 0), stop=(k == NK - 1))
                        ob = ysb_pool.tile([P, OC], fp32)
                        nc.scalar.activation(
                            ob[:], ps[:], mybir.ActivationFunctionType.Copy,
                            scale=0.5)
                        nc.sync.dma_start(
                            out=y_out[s * P:(s + 1) * P,
                                      oc * OC:(oc + 1) * OC],
                            in_=ob[:])

    _hoist_waits(nc)
    return nc


_program_cache = None


def _get_program():
    global _program_cache
    if _program_cache is None:
        _program_cache = build_program()
    return _program_cache


def run(x, weight, trace=False, trace_cores=None):
    x = np.asarray(x, dtype=np.float32)
    weight = np.asarray(weight, dtype=np.float32)
    assert x.shape == (B, S, I), x.shape
    assert weight.shape == (O, I), weight.shape
    nc = _get_program()
    wT = np.ascontiguousarray(weight.T)           # [I, O]
    in_maps = []
    for c in range(B):
        in_maps.append({
            "xT": np.ascontiguousarray(x[c].T),   # [I, S]
            "wT": wT,
            "wp": wT[c * WPR:(c + 1) * WPR],      # contiguous row slice
        })
    kwargs = {}
    if trace_cores:
        kwargs["trace_cores"] = trace_cores
    res = run_bass_kernel_spmd(nc, in_maps, list(range(B)), trace=trace,
                               **kwargs)
    y = np.stack([res.results[c]["y"] for c in range(B)], axis=0)
    return y, res


def kernel(x, weight):
    y, _ = run(x, weight)
    return y
